# revision 20
# baseline (speedup 1.0000x reference)
"""Trainium2 Bass kernel for nn_CovaMLoss.

Computes sim[b,k,n] = sum_{c,d} qhat[b,c,n] * S[k,c,d] * qhat[b,d,n] where
qhat is the per-(b,c)-row L2-normalized input reshaped to [B, C, H*W], and
returns sim reshaped to [B, 1, K*H*W].

Strategy (per core, data-parallel over B across 8 cores):
  Host: symmetrize each S_k, eigendecompose, build W[c, (k,i)] = V_k[:,i] *
  sqrt(|lam_ki|) so that sim[k,n] = sum_i sign_ki * (W[:,ki] . qhat[:,n])^2.
  Device: P = W^T qhat via row-tiled (contract=32) PE matmuls into PSUM,
  square via ACT/DVE on the PSUM->SBUF drain, then reduce over i with
  sign-carrying mask matmuls (PSUM accumulation over slot groups).
  Row norms ride on an ACT Square+accum pass over q plus one tiny
  fold/replicate matmul; 1/norm is folded into per-batch scaled W.
"""

import sys

for _p in ("/opt/trn_rl_repo", "/root/.axon_site/_ro/trn_rl_repo"):
    if _p not in sys.path:
        sys.path.append(_p)

from contextlib import ExitStack

import numpy as np

import concourse.bass as bass  # noqa: F401  (bass must import before tile)
import concourse.tile as tile
from concourse import bacc, bass_utils, mybir

B, C, H, W, K = 64, 32, 64, 64, 16
N = H * W                  # 4096
NCORES = 8
BPC = B // NCORES          # 8 batches per core
S = 4                      # n-superblocks stacked on partitions
FPB = N // S               # 1024 free elems per s-block
CHUNK = 512                # matmul moving-operand chunk (one PSUM bank)
KC = K * C                 # 512 slots
G = KC // 128              # 4 slot groups of 128

F32 = mybir.dt.float32
F32R = mybir.dt.float32r
BF16 = mybir.dt.bfloat16
AF = mybir.ActivationFunctionType


def _host_prep(covas: np.ndarray):
    """Eigen-decompose symmetrized covas into sqrt-scaled directions."""
    Wmat = np.zeros((C, KC), np.float64)
    sign = np.zeros(KC, np.float64)
    for k in range(K):
        T = (covas[k].astype(np.float64) + covas[k].astype(np.float64).T) / 2.0
        lam, V = np.linalg.eigh(T)
        Wmat[:, k * C:(k + 1) * C] = V * np.sqrt(np.abs(lam))[None, :]
        sign[k * C:(k + 1) * C] = np.sign(lam)
    # W4[32*s + c, j] = W[c, j], replicated over the 4 s-blocks
    W4 = np.tile(Wmat.astype(np.float32), (S, 1))                  # [128, 512]
    # masks[j_local, 32*g + k] = sign for slot (128*g + j_local) when that
    # slot's k matches; 32 columns per group (16 real k's + 16 zeros so the
    # mask matmul initializes the full 32-partition sim stripe).
    masks = np.zeros((128, 32 * G), np.float32)  # cast to bf16 below
    for g in range(G):
        for j in range(128):
            slot = 128 * g + j
            masks[j, 32 * g + slot // C] = sign[slot]
    # foldrep[32*s + c, 32*s' + c'] = (c == c'): one matmul that both sums
    # the per-s-block partial norms and re-replicates to all 128 partitions.
    foldrep = np.tile(np.eye(C, dtype=np.float32), (S, S))         # [128, 128]
    import ml_dtypes
    return W4, masks.astype(ml_dtypes.bfloat16), foldrep


def _host_prep_v2(covas: np.ndarray):
    """Pair opposite-sign eigenvalues into products u.v = lam_p*y_p^2 +
    lam_m*y_m^2 for 128 slots (drained via DVE tensor_mul), keep the rest
    as plain sign-carrying squares (drained via ACT Square).

    Layout: w4 columns [0:128) = u (group 0), [128:384) = squares (groups
    1-2), [384:512) = v factors. masks [128, 96] = per-A-group 32-column
    sign masks."""
    import ml_dtypes
    A = np.zeros((C, 384), np.float64)
    Bm = np.zeros((C, 128), np.float64)
    pairs, squares = [], []
    for k in range(K):
        T = (covas[k].astype(np.float64) + covas[k].astype(np.float64).T) / 2.0
        lam, V = np.linalg.eigh(T)
        pos = sorted([i for i in range(C) if lam[i] > 0], key=lambda i: -lam[i])
        neg = sorted([i for i in range(C) if lam[i] <= 0], key=lambda i: lam[i])
        npair = min(len(pos), len(neg))
        for t in range(npair):
            pairs.append((k, lam[pos[t]], V[:, pos[t]], lam[neg[t]], V[:, neg[t]]))
        for i in pos[npair:] + neg[npair:]:
            squares.append((k, lam[i], V[:, i]))
    assert len(pairs) >= 128, f"only {len(pairs)} opposite-sign pairs"
    prod_k = np.zeros(128, np.int64)
    for j, (k, lp, vp, lm, vm) in enumerate(pairs[:128]):
        a = np.sqrt(lp) * vp
        bv = np.sqrt(-lm) * vm
        A[:, j] = a + bv
        Bm[:, j] = a - bv
        prod_k[j] = k
    for (k, lp, vp, lm, vm) in pairs[128:]:
        squares.append((k, lp, vp))
        squares.append((k, lm, vm))
    assert len(squares) == 256
    masks = np.zeros((128, 96), np.float32)
    for j in range(128):
        masks[j, prod_k[j]] = 1.0
    for j, (k, lam, v) in enumerate(squares):
        A[:, 128 + j] = np.sqrt(abs(lam)) * v
        g = 1 + j // 128
        masks[j % 128, 32 * g + k] = np.sign(lam)
    Wfull = np.concatenate([A, Bm], axis=1).astype(np.float32)   # [32, 512]
    W4 = np.tile(Wfull, (S, 1))                                  # [128, 512]
    foldrep = np.tile(np.eye(C, dtype=np.float32), (S, S))
    return W4, masks.astype(ml_dtypes.bfloat16), foldrep


def _build_kernel(repeat: int = 1, drain_dve_set=None, variant: str = "v1"):
    nc = bacc.Bacc(
        "TRN2",
        target_bir_lowering=False,
        debug=False,
        enable_asserts=True,
        num_devices=NCORES,
    )
    q_ap = nc.dram_tensor("q", [BPC, C, N], F32R, kind="ExternalInput").ap()
    w4_ap = nc.dram_tensor("w4", [128, KC], F32, kind="ExternalInput").ap()
    n_mask_g = 3 if variant == "v2" else G
    mk_ap = nc.dram_tensor("masks", [128, 32 * n_mask_g], BF16, kind="ExternalInput").ap()
    fr_ap = nc.dram_tensor("foldrep", [128, 128], F32, kind="ExternalInput").ap()
    # Raw stage dumps [b, m, 128, 512]; host unshuffles (k,s,m) -> [b, k, n].
    out_ap = nc.dram_tensor(
        "sim_raw", [BPC, FPB // CHUNK, 128, CHUNK], F32, kind="ExternalOutput"
    ).ap()

    with tile.TileContext(nc) as tc, ExitStack() as ctx:
        const = ctx.enter_context(tc.tile_pool(name="const", bufs=1))
        qpool = ctx.enter_context(tc.tile_pool(name="qpool", bufs=2))
        scr_pool = ctx.enter_context(tc.tile_pool(name="scr", bufs=2))
        nrm_pool = ctx.enter_context(tc.tile_pool(name="nrm", bufs=4))
        wb_pool = ctx.enter_context(tc.tile_pool(name="wb", bufs=2))
        p2_pool = ctx.enter_context(tc.tile_pool(name="p2", bufs=4))
        stage_pool = ctx.enter_context(tc.tile_pool(name="stage", bufs=2))
        tmp_pool = ctx.enter_context(tc.tile_pool(name="tmp", bufs=2))
        psA = ctx.enter_context(tc.tile_pool(name="psA", bufs=2, space="PSUM"))
        psSim = ctx.enter_context(tc.tile_pool(name="psSim", bufs=2, space="PSUM"))
        psNrm = ctx.enter_context(tc.tile_pool(name="psNrm", bufs=1, space="PSUM"))
        psB = (ctx.enter_context(tc.tile_pool(name="psB", bufs=1, space="PSUM"))
               if variant == "v2" else None)

        w4 = const.tile([128, KC], F32)
        nc.sync.dma_start(w4[:], w4_ap[:])
        masks = const.tile([128, 32 * n_mask_g], BF16)
        nc.sync.dma_start(masks[:], mk_ap[:])
        foldrep = const.tile([128, 128], F32)
        nc.sync.dma_start(foldrep[:], fr_ap[:])

        # Round-robin the PSUM->SBUF square-drain between ACT and DVE.
        # ACT tile = 997ns, DVE tile = ~2258ns; ratio ~ 11:5 per 16 tiles.
        # Empirical: keeping the whole PSUM->SBUF square-drain on ACT beats
        # an ACT/DVE split (DVE needs a copy+mul pair per tile and its DRAINs
        # lengthen the drain->mask-matmul chain).
        drain_dve = set() if drain_dve_set is None else drain_dve_set

        for b_iter in range(BPC * repeat):
            b = b_iter % BPC
            q4 = qpool.tile([128, FPB], F32R)
            nc.sync.dma_start(q4[:], q_ap[b].rearrange("c (s f) -> s c f", s=S))

            # ---- row norms -> rnorm4 [128, 1] (1/norm, replicated per s) --
            scr = scr_pool.tile([128, FPB], F32)
            ss4 = nrm_pool.tile([128, 1], F32)
            nc.scalar.activation(scr[:], q4.bitcast(F32)[:], AF.Square, accum_out=ss4[:])
            if variant == "v2":
                nrm2 = psB.tile([128, 1], F32, tag="bps")
            else:
                nrm2 = psNrm.tile([128, 1], F32)
            nc.tensor.matmul(nrm2[:], lhsT=foldrep[:], rhs=ss4[:],
                             start=True, stop=True)
            snrm = nrm_pool.tile([128, 1], F32)
            nc.scalar.activation(snrm[:], nrm2[:], AF.Sqrt)
            rnorm = nrm_pool.tile([128, 1], F32)
            nc.vector.reciprocal(rnorm[:], snrm[:])
            wb = wb_pool.tile([128, KC], F32R)
            nc.vector.tensor_scalar_mul(wb[:], w4[:], rnorm[:])

            # ---- main pipeline ----
            if variant == "v2":
                # group 0 = paired products (DVE tensor_mul of A-psum x
                # B-sbuf); groups 1-2 = plain squares (ACT). B factors sit in
                # wb columns [384:512).
                for m in range(FPB // CHUNK):
                    sim_ps = psSim.tile([128, CHUNK], F32)
                    for half in range(2):
                        b_ps = psB.tile([128, 2 * CHUNK], F32, tag="bps")
                        a_ps = psA.tile([128, 2 * CHUNK], F32, tag="aps")
                        for si in range(2):
                            s = 2 * half + si
                            nc.tensor.matmul(
                                b_ps[:, si * CHUNK:(si + 1) * CHUNK],
                                lhsT=wb[32 * s:32 * (s + 1), 384:512],
                                rhs=q4[32 * s:32 * (s + 1),
                                       m * CHUNK:(m + 1) * CHUNK],
                                start=True, stop=True,
                                tile_position=(32 * s, 0),
                            )
                            nc.tensor.matmul(
                                a_ps[:, si * CHUNK:(si + 1) * CHUNK],
                                lhsT=wb[32 * s:32 * (s + 1), 0:128],
                                rhs=q4[32 * s:32 * (s + 1),
                                       m * CHUNK:(m + 1) * CHUNK],
                                start=True, stop=True,
                                tile_position=(32 * s, 0),
                            )
                        bsb = tmp_pool.tile([128, 2 * CHUNK], F32, tag="bsb")
                        if half == 0:
                            nc.scalar.activation(bsb[:], b_ps[:], AF.Copy)
                        else:
                            nc.vector.tensor_copy(bsb[:], b_ps[:])
                        prod = p2_pool.tile([128, 2 * CHUNK], BF16, tag="p2")
                        nc.vector.tensor_mul(prod[:], a_ps[:], bsb[:])
                        for si in range(2):
                            s = 2 * half + si
                            nc.tensor.matmul(
                                sim_ps[32 * s:32 * (s + 1), :],
                                lhsT=masks[:, 0:32],
                                rhs=prod[:, si * CHUNK:(si + 1) * CHUNK],
                                start=True, stop=False,
                                tile_position=(0, 32 * s),
                                skip_group_check=True,
                            )
                    for g in (1, 2):
                        for half in range(2):
                            a_ps = psA.tile([128, 2 * CHUNK], F32, tag="aps")
                            for si in range(2):
                                s = 2 * half + si
                                nc.tensor.matmul(
                                    a_ps[:, si * CHUNK:(si + 1) * CHUNK],
                                    lhsT=wb[32 * s:32 * (s + 1),
                                            128 * g:128 * (g + 1)],
                                    rhs=q4[32 * s:32 * (s + 1),
                                           m * CHUNK:(m + 1) * CHUNK],
                                    start=True, stop=True,
                                    tile_position=(32 * s, 0),
                                )
                            p2 = p2_pool.tile([128, 2 * CHUNK], BF16, tag="p2")
                            nc.scalar.activation(p2[:], a_ps[:], AF.Square)
                            for si in range(2):
                                s = 2 * half + si
                                nc.tensor.matmul(
                                    sim_ps[32 * s:32 * (s + 1), :],
                                    lhsT=masks[:, 32 * g:32 * (g + 1)],
                                    rhs=p2[:, si * CHUNK:(si + 1) * CHUNK],
                                    start=False, stop=(g == 2),
                                    tile_position=(0, 32 * s),
                                    skip_group_check=True,
                                )
                    stage = stage_pool.tile([128, CHUNK], F32)
                    nc.vector.tensor_copy(stage[:], sim_ps[:])
                    nc.sync.dma_start(out_ap[b, m], stage[:])
                continue
            for m in range(FPB // CHUNK):          # 2 chunks per s-block
                sim_ps = psSim.tile([128, CHUNK], F32)
                di = 0
                for g in range(G):
                    for half in range(2):          # s-pairs (0,1), (2,3)
                        a_ps = psA.tile([128, 2 * CHUNK], F32)   # 2 banks
                        for si in range(2):
                            s = 2 * half + si
                            nc.tensor.matmul(
                                a_ps[:, si * CHUNK:(si + 1) * CHUNK],
                                lhsT=wb[32 * s:32 * (s + 1),
                                        128 * g:128 * (g + 1)],
                                rhs=q4[32 * s:32 * (s + 1),
                                       m * CHUNK:(m + 1) * CHUNK],
                                start=True, stop=True,
                                tile_position=(32 * s, 0),
                            )
                        p2 = p2_pool.tile([128, 2 * CHUNK], BF16)
                        if di in drain_dve:
                            # DVE can't read two PSUM operands: copy out first.
                            tmp = tmp_pool.tile([128, 2 * CHUNK], F32)
                            nc.vector.tensor_copy(tmp[:], a_ps[:])
                            nc.vector.tensor_mul(p2[:], tmp[:], tmp[:])
                        else:
                            nc.scalar.activation(p2[:], a_ps[:], AF.Square)
                        di += 1
                        for si in range(2):
                            s = 2 * half + si
                            nc.tensor.matmul(
                                sim_ps[32 * s:32 * (s + 1), :],
                                lhsT=masks[:, 32 * g:32 * (g + 1)],
                                rhs=p2[:, si * CHUNK:(si + 1) * CHUNK],
                                start=(g == 0), stop=(g == G - 1),
                                tile_position=(0, 32 * s),
                                skip_group_check=True,
                            )
                stage = stage_pool.tile([128, CHUNK], F32)
                nc.vector.tensor_copy(stage[:], sim_ps[:])
                # raw[b, m, 32*s + k, f] = sim[b, k, 1024*s + 512*m + f]
                nc.sync.dma_start(out_ap[b, m], stage[:])
    nc.compile()
    return nc


_CACHE = {}


VARIANT = "v2"


def _get_nc(repeat: int = 1, drain_dve_set=None, variant=None):
    variant = VARIANT if variant is None else variant
    key = ("nc", repeat, None if drain_dve_set is None else tuple(sorted(drain_dve_set)), variant)
    if key not in _CACHE:
        _CACHE[key] = _build_kernel(repeat, drain_dve_set, variant)
    return _CACHE[key]


def make_in_maps(input_np: np.ndarray, covas_np: np.ndarray, variant=None):
    variant = VARIANT if variant is None else variant
    q = np.ascontiguousarray(
        np.asarray(input_np, dtype=np.float32).reshape(B, C, N))
    prep = _host_prep_v2 if variant == "v2" else _host_prep
    W4, masks, foldrep = prep(np.asarray(covas_np, dtype=np.float32))
    in_maps = []
    for c in range(NCORES):
        in_maps.append({
            "q": np.ascontiguousarray(q[c * BPC:(c + 1) * BPC]),
            "w4": W4,
            "masks": masks,
            "foldrep": foldrep,
        })
    return in_maps


def assemble(results) -> np.ndarray:
    out = np.empty((B, K, N), np.float32)
    for c in range(NCORES):
        raw = results[c]["sim_raw"]                 # [BPC, 2, 128, 512]
        # raw[b, m, 32*s + k, f] -> sim[b, k, 1024*s + 512*m + f]
        r = raw.reshape(BPC, FPB // CHUNK, S, 32, CHUNK)[:, :, :, :K, :]
        out[c * BPC:(c + 1) * BPC] = (
            r.transpose(0, 3, 2, 1, 4).reshape(BPC, K, N))
    return np.ascontiguousarray(out.reshape(B, 1, K * N))


def _pick_variant(covas_np: np.ndarray) -> str:
    """v2 needs >=128 opposite-sign eigenvalue pairs across the K covas
    (always true for generic inputs); fall back to v1 otherwise."""
    total = 0
    for k in range(K):
        T = (covas_np[k].astype(np.float64) + covas_np[k].astype(np.float64).T) / 2
        lam = np.linalg.eigvalsh(T)
        total += min(int((lam > 0).sum()), int((lam <= 0).sum()))
    return "v2" if total >= 128 else "v1"


def kernel(input: np.ndarray, support_covas: np.ndarray) -> np.ndarray:
    covas = np.asarray(support_covas, dtype=np.float32)
    variant = _pick_variant(covas)
    nc = _get_nc(variant=variant)
    in_maps = make_in_maps(input, covas, variant=variant)
    res = bass_utils.run_bass_kernel_spmd(nc, in_maps, core_ids=list(range(NCORES)))
    return assemble(res.results)


if __name__ == "__main__":
    rng = np.random.default_rng(0)
    inp = rng.standard_normal((B, C, H, W)).astype(np.float32)
    cov = rng.standard_normal((K, C, C)).astype(np.float32)
    out = kernel(inp, cov)
    print("kernel output shape:", out.shape, out.dtype)


# revision 23
# speedup vs baseline: 1.2503x; 1.2503x over previous
"""Trainium2 Bass kernel for nn_CovaMLoss.

Computes sim[b,k,n] = sum_{c,d} qhat[b,c,n] * S[k,c,d] * qhat[b,d,n] where
qhat is the per-(b,c)-row L2-normalized input reshaped to [B, C, H*W], and
returns sim reshaped to [B, 1, K*H*W].

Strategy (per core, data-parallel over B across 8 cores):
  Host: symmetrize each S_k, eigendecompose, build W[c, (k,i)] = V_k[:,i] *
  sqrt(|lam_ki|) so that sim[k,n] = sum_i sign_ki * (W[:,ki] . qhat[:,n])^2.
  Device: P = W^T qhat via row-tiled (contract=32) PE matmuls into PSUM,
  square via ACT/DVE on the PSUM->SBUF drain, then reduce over i with
  sign-carrying mask matmuls (PSUM accumulation over slot groups).
  Row norms ride on an ACT Square+accum pass over q plus one tiny
  fold/replicate matmul; 1/norm is folded into per-batch scaled W.
"""

import sys

for _p in ("/opt/trn_rl_repo", "/root/.axon_site/_ro/trn_rl_repo"):
    if _p not in sys.path:
        sys.path.append(_p)

from contextlib import ExitStack

import numpy as np

import concourse.bass as bass  # noqa: F401  (bass must import before tile)
import concourse.tile as tile
from concourse import bacc, bass_utils, mybir

B, C, H, W, K = 64, 32, 64, 64, 16
N = H * W                  # 4096
NCORES = 8
BPC = B // NCORES          # 8 batches per core
S = 4                      # n-superblocks stacked on partitions
FPB = N // S               # 1024 free elems per s-block
CHUNK = 512                # matmul moving-operand chunk (one PSUM bank)
KC = K * C                 # 512 slots
G = KC // 128              # 4 slot groups of 128

F32 = mybir.dt.float32
F32R = mybir.dt.float32r
BF16 = mybir.dt.bfloat16
AF = mybir.ActivationFunctionType


def _host_prep(covas: np.ndarray):
    """Eigen-decompose symmetrized covas into sqrt-scaled directions."""
    Wmat = np.zeros((C, KC), np.float64)
    sign = np.zeros(KC, np.float64)
    for k in range(K):
        T = (covas[k].astype(np.float64) + covas[k].astype(np.float64).T) / 2.0
        lam, V = np.linalg.eigh(T)
        Wmat[:, k * C:(k + 1) * C] = V * np.sqrt(np.abs(lam))[None, :]
        sign[k * C:(k + 1) * C] = np.sign(lam)
    # W4[32*s + c, j] = W[c, j], replicated over the 4 s-blocks
    W4 = np.tile(Wmat.astype(np.float32), (S, 1))                  # [128, 512]
    # masks[j_local, 32*g + k] = sign for slot (128*g + j_local) when that
    # slot's k matches; 32 columns per group (16 real k's + 16 zeros so the
    # mask matmul initializes the full 32-partition sim stripe).
    masks = np.zeros((128, 32 * G), np.float32)  # cast to bf16 below
    for g in range(G):
        for j in range(128):
            slot = 128 * g + j
            masks[j, 32 * g + slot // C] = sign[slot]
    # foldrep[32*s + c, 32*s' + c'] = (c == c'): one matmul that both sums
    # the per-s-block partial norms and re-replicates to all 128 partitions.
    foldrep = np.tile(np.eye(C, dtype=np.float32), (S, S))         # [128, 128]
    import ml_dtypes
    return W4, masks.astype(ml_dtypes.bfloat16), foldrep


def _host_prep_v2(covas: np.ndarray):
    """Pair opposite-sign eigenvalues into products u.v = lam_p*y_p^2 +
    lam_m*y_m^2 for 128 slots (drained via DVE tensor_mul), keep the rest
    as plain sign-carrying squares (drained via ACT Square).

    Layout: w4 columns [0:128) = u (group 0), [128:384) = squares (groups
    1-2), [384:512) = v factors. masks [128, 96] = per-A-group 32-column
    sign masks."""
    import ml_dtypes
    A = np.zeros((C, 384), np.float64)
    Bm = np.zeros((C, 128), np.float64)
    pairs, squares = [], []
    for k in range(K):
        T = (covas[k].astype(np.float64) + covas[k].astype(np.float64).T) / 2.0
        lam, V = np.linalg.eigh(T)
        pos = sorted([i for i in range(C) if lam[i] > 0], key=lambda i: -lam[i])
        neg = sorted([i for i in range(C) if lam[i] <= 0], key=lambda i: lam[i])
        npair = min(len(pos), len(neg))
        for t in range(npair):
            pairs.append((k, lam[pos[t]], V[:, pos[t]], lam[neg[t]], V[:, neg[t]]))
        for i in pos[npair:] + neg[npair:]:
            squares.append((k, lam[i], V[:, i]))
    assert len(pairs) >= 128, f"only {len(pairs)} opposite-sign pairs"
    prod_k = np.zeros(128, np.int64)
    for j, (k, lp, vp, lm, vm) in enumerate(pairs[:128]):
        a = np.sqrt(lp) * vp
        bv = np.sqrt(-lm) * vm
        A[:, j] = a + bv
        Bm[:, j] = a - bv
        prod_k[j] = k
    for (k, lp, vp, lm, vm) in pairs[128:]:
        squares.append((k, lp, vp))
        squares.append((k, lm, vm))
    assert len(squares) == 256
    masks = np.zeros((128, 96), np.float32)
    for j in range(128):
        masks[j, prod_k[j]] = 1.0
    for j, (k, lam, v) in enumerate(squares):
        A[:, 128 + j] = np.sqrt(abs(lam)) * v
        g = 1 + j // 128
        masks[j % 128, 32 * g + k] = np.sign(lam)
    Wfull = np.concatenate([A, Bm], axis=1).astype(np.float32)   # [32, 512]
    W4 = np.tile(Wfull, (S, 1))                                  # [128, 512]
    foldrep = np.tile(np.eye(C, dtype=np.float32), (S, S))
    return W4, masks.astype(ml_dtypes.bfloat16), foldrep


def _build_kernel(repeat: int = 1, drain_dve_set=None, variant: str = "v1"):
    nc = bacc.Bacc(
        "TRN2",
        target_bir_lowering=False,
        debug=False,
        enable_asserts=True,
        num_devices=NCORES,
    )
    q_ap = nc.dram_tensor("q", [BPC, C, N], F32R, kind="ExternalInput").ap()
    w4_ap = nc.dram_tensor("w4", [128, KC], F32, kind="ExternalInput").ap()
    n_mask_g = 3 if variant == "v2" else G
    mk_ap = nc.dram_tensor("masks", [128, 32 * n_mask_g], BF16, kind="ExternalInput").ap()
    fr_ap = nc.dram_tensor("foldrep", [128, 128], F32, kind="ExternalInput").ap()
    # Raw stage dumps [b, m, 128, 512]; host unshuffles (k,s,m) -> [b, k, n].
    out_ap = nc.dram_tensor(
        "sim_raw", [BPC, FPB // CHUNK, 128, CHUNK], F32, kind="ExternalOutput"
    ).ap()

    with tile.TileContext(nc) as tc, ExitStack() as ctx:
        const = ctx.enter_context(tc.tile_pool(name="const", bufs=1))
        qpool = ctx.enter_context(tc.tile_pool(name="qpool", bufs=2))
        scr_pool = ctx.enter_context(tc.tile_pool(name="scr", bufs=2))
        nrm_pool = ctx.enter_context(tc.tile_pool(name="nrm", bufs=4))
        wb_pool = ctx.enter_context(tc.tile_pool(name="wb", bufs=2))
        p2_pool = ctx.enter_context(tc.tile_pool(name="p2", bufs=6))
        stage_pool = ctx.enter_context(tc.tile_pool(name="stage", bufs=3))
        tmp_pool = ctx.enter_context(tc.tile_pool(name="tmp", bufs=4))
        psA = ctx.enter_context(tc.tile_pool(name="psA", bufs=2, space="PSUM"))
        psSim = ctx.enter_context(tc.tile_pool(name="psSim", bufs=2, space="PSUM"))
        psNrm = ctx.enter_context(tc.tile_pool(name="psNrm", bufs=1, space="PSUM"))
        psB = (ctx.enter_context(tc.tile_pool(name="psB", bufs=1, space="PSUM"))
               if variant == "v2" else None)

        w4 = const.tile([128, KC], F32)
        nc.sync.dma_start(w4[:], w4_ap[:])
        masks = const.tile([128, 32 * n_mask_g], BF16)
        nc.sync.dma_start(masks[:], mk_ap[:])
        foldrep = const.tile([128, 128], F32)
        nc.sync.dma_start(foldrep[:], fr_ap[:])

        # Round-robin the PSUM->SBUF square-drain between ACT and DVE.
        # ACT tile = 997ns, DVE tile = ~2258ns; ratio ~ 11:5 per 16 tiles.
        # Empirical: keeping the whole PSUM->SBUF square-drain on ACT beats
        # an ACT/DVE split (DVE needs a copy+mul pair per tile and its DRAINs
        # lengthen the drain->mask-matmul chain).
        drain_dve = set() if drain_dve_set is None else drain_dve_set

        for b_iter in range(BPC * repeat):
            b = b_iter % BPC
            q4 = qpool.tile([128, FPB], F32R)
            nc.sync.dma_start(q4[:], q_ap[b].rearrange("c (s f) -> s c f", s=S))

            # ---- row norms -> rnorm4 [128, 1] (1/norm, replicated per s) --
            scr = scr_pool.tile([128, FPB], F32)
            ss4 = nrm_pool.tile([128, 1], F32)
            nc.scalar.activation(scr[:], q4.bitcast(F32)[:], AF.Square, accum_out=ss4[:])
            if variant == "v2":
                nrm2 = psB.tile([128, 1], F32, tag="bps")
            else:
                nrm2 = psNrm.tile([128, 1], F32)
            nc.tensor.matmul(nrm2[:], lhsT=foldrep[:], rhs=ss4[:],
                             start=True, stop=True)
            snrm = nrm_pool.tile([128, 1], F32)
            nc.scalar.activation(snrm[:], nrm2[:], AF.Sqrt)
            rnorm = nrm_pool.tile([128, 1], F32)
            nc.vector.reciprocal(rnorm[:], snrm[:])
            wb = wb_pool.tile([128, KC], F32R)
            nc.vector.tensor_scalar_mul(wb[:], w4[:], rnorm[:])

            # ---- main pipeline ----
            if variant == "v2":
                # group 0 = paired products (DVE tensor_mul of A-psum x
                # B-sbuf); groups 1-2 = plain squares (ACT). B factors sit in
                # wb columns [384:512). Coarse [128, 1024] PSUM tiles + mask
                # matmuls batched after each drain: a finer per-s interleave
                # (single-bank tiles, mask-mm right after each product)
                # measured 2.6x SLOWER on HW -- the dependent mask-matmuls
                # gate the PE's in-order stream on ACT/DVE at every step.
                for m in range(FPB // CHUNK):
                    sim_ps = psSim.tile([128, CHUNK], F32)
                    for half in range(2):
                        b_ps = psB.tile([128, 2 * CHUNK], F32, tag="bps")
                        a_ps = psA.tile([128, 2 * CHUNK], F32, tag="aps")
                        for si in range(2):
                            s = 2 * half + si
                            nc.tensor.matmul(
                                b_ps[:, si * CHUNK:(si + 1) * CHUNK],
                                lhsT=wb[32 * s:32 * (s + 1), 384:512],
                                rhs=q4[32 * s:32 * (s + 1),
                                       m * CHUNK:(m + 1) * CHUNK],
                                start=True, stop=True,
                                tile_position=(32 * s, 0),
                            )
                            nc.tensor.matmul(
                                a_ps[:, si * CHUNK:(si + 1) * CHUNK],
                                lhsT=wb[32 * s:32 * (s + 1), 0:128],
                                rhs=q4[32 * s:32 * (s + 1),
                                       m * CHUNK:(m + 1) * CHUNK],
                                start=True, stop=True,
                                tile_position=(32 * s, 0),
                            )
                        bsb = tmp_pool.tile([128, 2 * CHUNK], F32, tag="bsb")
                        if half == 0:
                            nc.scalar.activation(bsb[:], b_ps[:], AF.Copy)
                        else:
                            nc.vector.tensor_copy(bsb[:], b_ps[:])
                        prod = p2_pool.tile([128, 2 * CHUNK], BF16, tag="p2")
                        nc.vector.tensor_mul(prod[:], a_ps[:], bsb[:])
                        for si in range(2):
                            s = 2 * half + si
                            nc.tensor.matmul(
                                sim_ps[32 * s:32 * (s + 1), :],
                                lhsT=masks[:, 0:32],
                                rhs=prod[:, si * CHUNK:(si + 1) * CHUNK],
                                start=True, stop=False,
                                tile_position=(0, 32 * s),
                                skip_group_check=True,
                            )
                    for g in (1, 2):
                        for half in range(2):
                            a_ps = psA.tile([128, 2 * CHUNK], F32, tag="aps")
                            for si in range(2):
                                s = 2 * half + si
                                nc.tensor.matmul(
                                    a_ps[:, si * CHUNK:(si + 1) * CHUNK],
                                    lhsT=wb[32 * s:32 * (s + 1),
                                            128 * g:128 * (g + 1)],
                                    rhs=q4[32 * s:32 * (s + 1),
                                           m * CHUNK:(m + 1) * CHUNK],
                                    start=True, stop=True,
                                    tile_position=(32 * s, 0),
                                )
                            p2 = p2_pool.tile([128, 2 * CHUNK], BF16, tag="p2")
                            nc.scalar.activation(p2[:], a_ps[:], AF.Square)
                            for si in range(2):
                                s = 2 * half + si
                                nc.tensor.matmul(
                                    sim_ps[32 * s:32 * (s + 1), :],
                                    lhsT=masks[:, 32 * g:32 * (g + 1)],
                                    rhs=p2[:, si * CHUNK:(si + 1) * CHUNK],
                                    start=False, stop=(g == 2),
                                    tile_position=(0, 32 * s),
                                    skip_group_check=True,
                                )
                    stage = stage_pool.tile([128, CHUNK], F32)
                    nc.vector.tensor_copy(stage[:], sim_ps[:])
                    nc.sync.dma_start(out_ap[b, m], stage[:])
                continue
            for m in range(FPB // CHUNK):          # 2 chunks per s-block
                sim_ps = psSim.tile([128, CHUNK], F32)
                di = 0
                for g in range(G):
                    for half in range(2):          # s-pairs (0,1), (2,3)
                        a_ps = psA.tile([128, 2 * CHUNK], F32)   # 2 banks
                        for si in range(2):
                            s = 2 * half + si
                            nc.tensor.matmul(
                                a_ps[:, si * CHUNK:(si + 1) * CHUNK],
                                lhsT=wb[32 * s:32 * (s + 1),
                                        128 * g:128 * (g + 1)],
                                rhs=q4[32 * s:32 * (s + 1),
                                       m * CHUNK:(m + 1) * CHUNK],
                                start=True, stop=True,
                                tile_position=(32 * s, 0),
                            )
                        p2 = p2_pool.tile([128, 2 * CHUNK], BF16)
                        if di in drain_dve:
                            # DVE can't read two PSUM operands: copy out first.
                            tmp = tmp_pool.tile([128, 2 * CHUNK], F32)
                            nc.vector.tensor_copy(tmp[:], a_ps[:])
                            nc.vector.tensor_mul(p2[:], tmp[:], tmp[:])
                        else:
                            nc.scalar.activation(p2[:], a_ps[:], AF.Square)
                        di += 1
                        for si in range(2):
                            s = 2 * half + si
                            nc.tensor.matmul(
                                sim_ps[32 * s:32 * (s + 1), :],
                                lhsT=masks[:, 32 * g:32 * (g + 1)],
                                rhs=p2[:, si * CHUNK:(si + 1) * CHUNK],
                                start=(g == 0), stop=(g == G - 1),
                                tile_position=(0, 32 * s),
                                skip_group_check=True,
                            )
                stage = stage_pool.tile([128, CHUNK], F32)
                nc.vector.tensor_copy(stage[:], sim_ps[:])
                # raw[b, m, 32*s + k, f] = sim[b, k, 1024*s + 512*m + f]
                nc.sync.dma_start(out_ap[b, m], stage[:])
    nc.compile()
    return nc


_CACHE = {}


VARIANT = "v2"


def _get_nc(repeat: int = 1, drain_dve_set=None, variant=None):
    variant = VARIANT if variant is None else variant
    key = ("nc", repeat, None if drain_dve_set is None else tuple(sorted(drain_dve_set)), variant)
    if key not in _CACHE:
        _CACHE[key] = _build_kernel(repeat, drain_dve_set, variant)
    return _CACHE[key]


def make_in_maps(input_np: np.ndarray, covas_np: np.ndarray, variant=None):
    variant = VARIANT if variant is None else variant
    q = np.ascontiguousarray(
        np.asarray(input_np, dtype=np.float32).reshape(B, C, N))
    prep = _host_prep_v2 if variant == "v2" else _host_prep
    W4, masks, foldrep = prep(np.asarray(covas_np, dtype=np.float32))
    in_maps = []
    for c in range(NCORES):
        in_maps.append({
            "q": np.ascontiguousarray(q[c * BPC:(c + 1) * BPC]),
            "w4": W4,
            "masks": masks,
            "foldrep": foldrep,
        })
    return in_maps


def assemble(results) -> np.ndarray:
    out = np.empty((B, K, N), np.float32)
    for c in range(NCORES):
        raw = results[c]["sim_raw"]                 # [BPC, 2, 128, 512]
        # raw[b, m, 32*s + k, f] -> sim[b, k, 1024*s + 512*m + f]
        r = raw.reshape(BPC, FPB // CHUNK, S, 32, CHUNK)[:, :, :, :K, :]
        out[c * BPC:(c + 1) * BPC] = (
            r.transpose(0, 3, 2, 1, 4).reshape(BPC, K, N))
    return np.ascontiguousarray(out.reshape(B, 1, K * N))


def _pick_variant(covas_np: np.ndarray) -> str:
    """v2 needs >=128 opposite-sign eigenvalue pairs across the K covas
    (always true for generic inputs); fall back to v1 otherwise."""
    total = 0
    for k in range(K):
        T = (covas_np[k].astype(np.float64) + covas_np[k].astype(np.float64).T) / 2
        lam = np.linalg.eigvalsh(T)
        total += min(int((lam > 0).sum()), int((lam <= 0).sum()))
    return "v2" if total >= 128 else "v1"


def kernel(input: np.ndarray, support_covas: np.ndarray) -> np.ndarray:
    covas = np.asarray(support_covas, dtype=np.float32)
    variant = _pick_variant(covas)
    nc = _get_nc(variant=variant)
    in_maps = make_in_maps(input, covas, variant=variant)
    res = bass_utils.run_bass_kernel_spmd(nc, in_maps, core_ids=list(range(NCORES)))
    return assemble(res.results)


if __name__ == "__main__":
    rng = np.random.default_rng(0)
    inp = rng.standard_normal((B, C, H, W)).astype(np.float32)
    cov = rng.standard_normal((K, C, C)).astype(np.float32)
    out = kernel(inp, cov)
    print("kernel output shape:", out.shape, out.dtype)


# revision 24
# speedup vs baseline: 2.1264x; 1.7007x over previous
"""Trainium2 Bass kernel for nn_CovaMLoss.

Computes sim[b,k,n] = sum_{c,d} qhat[b,c,n] * S[k,c,d] * qhat[b,d,n] where
qhat is the per-(b,c)-row L2-normalized input reshaped to [B, C, H*W], and
returns sim reshaped to [B, 1, K*H*W].

Strategy (per core, data-parallel over B across 8 cores):
  Host: symmetrize each S_k, eigendecompose, build W[c, (k,i)] = V_k[:,i] *
  sqrt(|lam_ki|) so that sim[k,n] = sum_i sign_ki * (W[:,ki] . qhat[:,n])^2.
  Device: P = W^T qhat via row-tiled (contract=32) PE matmuls into PSUM,
  square via ACT/DVE on the PSUM->SBUF drain, then reduce over i with
  sign-carrying mask matmuls (PSUM accumulation over slot groups).
  Row norms ride on an ACT Square+accum pass over q plus one tiny
  fold/replicate matmul; 1/norm is folded into per-batch scaled W.
"""

import sys

for _p in ("/opt/trn_rl_repo", "/root/.axon_site/_ro/trn_rl_repo"):
    if _p not in sys.path:
        sys.path.append(_p)

from contextlib import ExitStack

import numpy as np

import concourse.bass as bass  # noqa: F401  (bass must import before tile)
import concourse.tile as tile
from concourse import bacc, bass_utils, mybir

B, C, H, W, K = 64, 32, 64, 64, 16
N = H * W                  # 4096
NCORES = 8
BPC = B // NCORES          # 8 batches per core
S = 4                      # n-superblocks stacked on partitions
FPB = N // S               # 1024 free elems per s-block
CHUNK = 512                # matmul moving-operand chunk (one PSUM bank)
KC = K * C                 # 512 slots
G = KC // 128              # 4 slot groups of 128

F32 = mybir.dt.float32
F32R = mybir.dt.float32r
BF16 = mybir.dt.bfloat16
AF = mybir.ActivationFunctionType


def _host_prep(covas: np.ndarray):
    """Eigen-decompose symmetrized covas into sqrt-scaled directions."""
    Wmat = np.zeros((C, KC), np.float64)
    sign = np.zeros(KC, np.float64)
    for k in range(K):
        T = (covas[k].astype(np.float64) + covas[k].astype(np.float64).T) / 2.0
        lam, V = np.linalg.eigh(T)
        Wmat[:, k * C:(k + 1) * C] = V * np.sqrt(np.abs(lam))[None, :]
        sign[k * C:(k + 1) * C] = np.sign(lam)
    # W4[32*s + c, j] = W[c, j], replicated over the 4 s-blocks
    W4 = np.tile(Wmat.astype(np.float32), (S, 1))                  # [128, 512]
    # masks[j_local, 32*g + k] = sign for slot (128*g + j_local) when that
    # slot's k matches; 32 columns per group (16 real k's + 16 zeros so the
    # mask matmul initializes the full 32-partition sim stripe).
    masks = np.zeros((128, 32 * G), np.float32)  # cast to bf16 below
    for g in range(G):
        for j in range(128):
            slot = 128 * g + j
            masks[j, 32 * g + slot // C] = sign[slot]
    # foldrep[32*s + c, 32*s' + c'] = (c == c'): one matmul that both sums
    # the per-s-block partial norms and re-replicates to all 128 partitions.
    foldrep = np.tile(np.eye(C, dtype=np.float32), (S, S))         # [128, 128]
    import ml_dtypes
    return W4, masks.astype(ml_dtypes.bfloat16), foldrep


def _host_prep_v2(covas: np.ndarray):
    """Pair opposite-sign eigenvalues into products u.v = lam_p*y_p^2 +
    lam_m*y_m^2 for 128 slots (drained via DVE tensor_mul), keep the rest
    as plain sign-carrying squares (drained via ACT Square).

    Layout: w4 columns [0:128) = u (group 0), [128:384) = squares (groups
    1-2), [384:512) = v factors. masks [128, 96] = per-A-group 32-column
    sign masks."""
    import ml_dtypes
    A = np.zeros((C, 384), np.float64)
    Bm = np.zeros((C, 128), np.float64)
    pairs, squares = [], []
    for k in range(K):
        T = (covas[k].astype(np.float64) + covas[k].astype(np.float64).T) / 2.0
        lam, V = np.linalg.eigh(T)
        pos = sorted([i for i in range(C) if lam[i] > 0], key=lambda i: -lam[i])
        neg = sorted([i for i in range(C) if lam[i] <= 0], key=lambda i: lam[i])
        npair = min(len(pos), len(neg))
        for t in range(npair):
            pairs.append((k, lam[pos[t]], V[:, pos[t]], lam[neg[t]], V[:, neg[t]]))
        for i in pos[npair:] + neg[npair:]:
            squares.append((k, lam[i], V[:, i]))
    assert len(pairs) >= 128, f"only {len(pairs)} opposite-sign pairs"
    prod_k = np.zeros(128, np.int64)
    for j, (k, lp, vp, lm, vm) in enumerate(pairs[:128]):
        a = np.sqrt(lp) * vp
        bv = np.sqrt(-lm) * vm
        A[:, j] = a + bv
        Bm[:, j] = a - bv
        prod_k[j] = k
    for (k, lp, vp, lm, vm) in pairs[128:]:
        squares.append((k, lp, vp))
        squares.append((k, lm, vm))
    assert len(squares) == 256
    masks = np.zeros((128, 96), np.float32)
    for j in range(128):
        masks[j, prod_k[j]] = 1.0
    for j, (k, lam, v) in enumerate(squares):
        A[:, 128 + j] = np.sqrt(abs(lam)) * v
        g = 1 + j // 128
        masks[j % 128, 32 * g + k] = np.sign(lam)
    Wfull = np.concatenate([A, Bm], axis=1).astype(np.float32)   # [32, 512]
    W4 = np.tile(Wfull, (S, 1))                                  # [128, 512]
    foldrep = np.tile(np.eye(C, dtype=np.float32), (S, S))
    return W4, masks.astype(ml_dtypes.bfloat16), foldrep


def _build_kernel(repeat: int = 1, drain_dve_set=None, variant: str = "v1"):
    nc = bacc.Bacc(
        "TRN2",
        target_bir_lowering=False,
        debug=False,
        enable_asserts=True,
        num_devices=NCORES,
    )
    q_ap = nc.dram_tensor("q", [BPC, C, N], F32R, kind="ExternalInput").ap()
    w4_ap = nc.dram_tensor("w4", [128, KC], F32, kind="ExternalInput").ap()
    n_mask_g = 3 if variant == "v2" else G
    mk_ap = nc.dram_tensor("masks", [128, 32 * n_mask_g], BF16, kind="ExternalInput").ap()
    fr_ap = nc.dram_tensor("foldrep", [128, 128], F32, kind="ExternalInput").ap()
    # Raw stage dumps [b, m, 128, 512]; host unshuffles (k,s,m) -> [b, k, n].
    out_ap = nc.dram_tensor(
        "sim_raw", [BPC, FPB // CHUNK, 128, CHUNK], F32, kind="ExternalOutput"
    ).ap()

    with tile.TileContext(nc) as tc, ExitStack() as ctx:
        const = ctx.enter_context(tc.tile_pool(name="const", bufs=1))
        qpool = ctx.enter_context(tc.tile_pool(name="qpool", bufs=2))
        scr_pool = ctx.enter_context(tc.tile_pool(name="scr", bufs=2))
        nrm_pool = ctx.enter_context(tc.tile_pool(name="nrm", bufs=4))
        wb_pool = ctx.enter_context(tc.tile_pool(name="wb", bufs=2))
        p2_pool = ctx.enter_context(tc.tile_pool(name="p2", bufs=6))
        stage_pool = ctx.enter_context(tc.tile_pool(name="stage", bufs=3))
        tmp_pool = ctx.enter_context(tc.tile_pool(name="tmp", bufs=4))
        psA = ctx.enter_context(tc.tile_pool(name="psA", bufs=2, space="PSUM"))
        psSim = ctx.enter_context(tc.tile_pool(name="psSim", bufs=2, space="PSUM"))
        psNrm = ctx.enter_context(tc.tile_pool(name="psNrm", bufs=1, space="PSUM"))
        psB = (ctx.enter_context(tc.tile_pool(name="psB", bufs=1, space="PSUM"))
               if variant == "v2" else None)

        w4 = const.tile([128, KC], F32)
        nc.sync.dma_start(w4[:], w4_ap[:])
        masks = const.tile([128, 32 * n_mask_g], BF16)
        nc.sync.dma_start(masks[:], mk_ap[:])
        foldrep = const.tile([128, 128], F32)
        nc.sync.dma_start(foldrep[:], fr_ap[:])

        # Round-robin the PSUM->SBUF square-drain between ACT and DVE.
        # ACT tile = 997ns, DVE tile = ~2258ns; ratio ~ 11:5 per 16 tiles.
        # Empirical: keeping the whole PSUM->SBUF square-drain on ACT beats
        # an ACT/DVE split (DVE needs a copy+mul pair per tile and its DRAINs
        # lengthen the drain->mask-matmul chain).
        drain_dve = set() if drain_dve_set is None else drain_dve_set

        for b_iter in range(BPC * repeat):
            b = b_iter % BPC
            q4 = qpool.tile([128, FPB], F32R)
            nc.sync.dma_start(q4[:], q_ap[b].rearrange("c (s f) -> s c f", s=S))

            # ---- row norms -> rnorm4 [128, 1] (1/norm, replicated per s) --
            scr = scr_pool.tile([128, FPB], F32)
            ss4 = nrm_pool.tile([128, 1], F32)
            if variant == "v2":
                # keep ACT (the drain bottleneck) free: square+reduce on DVE
                nc.vector.tensor_mul(scr[:], q4.bitcast(F32)[:], q4.bitcast(F32)[:])
                nc.vector.tensor_reduce(ss4[:], scr[:], axis=mybir.AxisListType.X,
                                        op=mybir.AluOpType.add)
            else:
                nc.scalar.activation(scr[:], q4.bitcast(F32)[:], AF.Square,
                                     accum_out=ss4[:])
            if variant == "v2":
                nrm2 = psB.tile([128, 1], F32, tag="bps")
            else:
                nrm2 = psNrm.tile([128, 1], F32)
            nc.tensor.matmul(nrm2[:], lhsT=foldrep[:], rhs=ss4[:],
                             start=True, stop=True)
            snrm = nrm_pool.tile([128, 1], F32)
            nc.scalar.activation(snrm[:], nrm2[:], AF.Sqrt)
            rnorm = nrm_pool.tile([128, 1], F32)
            nc.vector.reciprocal(rnorm[:], snrm[:])
            wb = wb_pool.tile([128, KC], F32R)
            nc.vector.tensor_scalar_mul(wb[:], w4[:], rnorm[:])

            # ---- main pipeline ----
            if variant == "v2":
                # group 0 = paired products (DVE tensor_mul of A-psum x
                # B-sbuf); groups 1-2 = plain squares (ACT). B factors sit in
                # wb columns [384:512). Coarse [128, 1024] PSUM tiles + mask
                # matmuls batched after each drain: a finer per-s interleave
                # (single-bank tiles, mask-mm right after each product)
                # measured 2.6x SLOWER on HW -- the dependent mask-matmuls
                # gate the PE's in-order stream on ACT/DVE at every step.
                for m in range(FPB // CHUNK):
                    sim_ps = psSim.tile([128, CHUNK], F32)
                    for half in range(2):
                        b_ps = psB.tile([128, 2 * CHUNK], F32, tag="bps")
                        a_ps = psA.tile([128, 2 * CHUNK], F32, tag="aps")
                        for si in range(2):
                            s = 2 * half + si
                            nc.tensor.matmul(
                                b_ps[:, si * CHUNK:(si + 1) * CHUNK],
                                lhsT=wb[32 * s:32 * (s + 1), 384:512],
                                rhs=q4[32 * s:32 * (s + 1),
                                       m * CHUNK:(m + 1) * CHUNK],
                                start=True, stop=True,
                                tile_position=(32 * s, 0),
                            )
                            nc.tensor.matmul(
                                a_ps[:, si * CHUNK:(si + 1) * CHUNK],
                                lhsT=wb[32 * s:32 * (s + 1), 0:128],
                                rhs=q4[32 * s:32 * (s + 1),
                                       m * CHUNK:(m + 1) * CHUNK],
                                start=True, stop=True,
                                tile_position=(32 * s, 0),
                            )
                        bsb = tmp_pool.tile([128, 2 * CHUNK], F32, tag="bsb")
                        if half == 0:
                            nc.scalar.activation(bsb[:], b_ps[:], AF.Copy)
                        else:
                            nc.vector.tensor_copy(bsb[:], b_ps[:])
                        prod = p2_pool.tile([128, 2 * CHUNK], BF16, tag="p2")
                        nc.vector.tensor_mul(prod[:], a_ps[:], bsb[:])
                        for si in range(2):
                            s = 2 * half + si
                            nc.tensor.matmul(
                                sim_ps[32 * s:32 * (s + 1), :],
                                lhsT=masks[:, 0:32],
                                rhs=prod[:, si * CHUNK:(si + 1) * CHUNK],
                                start=True, stop=False,
                                tile_position=(0, 32 * s),
                                skip_group_check=True,
                            )
                    for g in (1, 2):
                        for half in range(2):
                            a_ps = psA.tile([128, 2 * CHUNK], F32, tag="aps")
                            for si in range(2):
                                s = 2 * half + si
                                nc.tensor.matmul(
                                    a_ps[:, si * CHUNK:(si + 1) * CHUNK],
                                    lhsT=wb[32 * s:32 * (s + 1),
                                            128 * g:128 * (g + 1)],
                                    rhs=q4[32 * s:32 * (s + 1),
                                           m * CHUNK:(m + 1) * CHUNK],
                                    start=True, stop=True,
                                    tile_position=(32 * s, 0),
                                )
                            p2 = p2_pool.tile([128, 2 * CHUNK], BF16, tag="p2")
                            nc.scalar.activation(p2[:], a_ps[:], AF.Square)
                            for si in range(2):
                                s = 2 * half + si
                                nc.tensor.matmul(
                                    sim_ps[32 * s:32 * (s + 1), :],
                                    lhsT=masks[:, 32 * g:32 * (g + 1)],
                                    rhs=p2[:, si * CHUNK:(si + 1) * CHUNK],
                                    start=False, stop=(g == 2),
                                    tile_position=(0, 32 * s),
                                    skip_group_check=True,
                                )
                    stage = stage_pool.tile([128, CHUNK], F32)
                    nc.vector.tensor_copy(stage[:], sim_ps[:])
                    nc.sync.dma_start(out_ap[b, m], stage[:])
                continue
            for m in range(FPB // CHUNK):          # 2 chunks per s-block
                sim_ps = psSim.tile([128, CHUNK], F32)
                di = 0
                for g in range(G):
                    for half in range(2):          # s-pairs (0,1), (2,3)
                        a_ps = psA.tile([128, 2 * CHUNK], F32)   # 2 banks
                        for si in range(2):
                            s = 2 * half + si
                            nc.tensor.matmul(
                                a_ps[:, si * CHUNK:(si + 1) * CHUNK],
                                lhsT=wb[32 * s:32 * (s + 1),
                                        128 * g:128 * (g + 1)],
                                rhs=q4[32 * s:32 * (s + 1),
                                       m * CHUNK:(m + 1) * CHUNK],
                                start=True, stop=True,
                                tile_position=(32 * s, 0),
                            )
                        p2 = p2_pool.tile([128, 2 * CHUNK], BF16)
                        if di in drain_dve:
                            # DVE can't read two PSUM operands: copy out first.
                            tmp = tmp_pool.tile([128, 2 * CHUNK], F32)
                            nc.vector.tensor_copy(tmp[:], a_ps[:])
                            nc.vector.tensor_mul(p2[:], tmp[:], tmp[:])
                        else:
                            nc.scalar.activation(p2[:], a_ps[:], AF.Square)
                        di += 1
                        for si in range(2):
                            s = 2 * half + si
                            nc.tensor.matmul(
                                sim_ps[32 * s:32 * (s + 1), :],
                                lhsT=masks[:, 32 * g:32 * (g + 1)],
                                rhs=p2[:, si * CHUNK:(si + 1) * CHUNK],
                                start=(g == 0), stop=(g == G - 1),
                                tile_position=(0, 32 * s),
                                skip_group_check=True,
                            )
                stage = stage_pool.tile([128, CHUNK], F32)
                nc.vector.tensor_copy(stage[:], sim_ps[:])
                # raw[b, m, 32*s + k, f] = sim[b, k, 1024*s + 512*m + f]
                nc.sync.dma_start(out_ap[b, m], stage[:])
    nc.compile()
    return nc


_CACHE = {}


VARIANT = "v2"


def _get_nc(repeat: int = 1, drain_dve_set=None, variant=None):
    variant = VARIANT if variant is None else variant
    key = ("nc", repeat, None if drain_dve_set is None else tuple(sorted(drain_dve_set)), variant)
    if key not in _CACHE:
        _CACHE[key] = _build_kernel(repeat, drain_dve_set, variant)
    return _CACHE[key]


def make_in_maps(input_np: np.ndarray, covas_np: np.ndarray, variant=None):
    variant = VARIANT if variant is None else variant
    q = np.ascontiguousarray(
        np.asarray(input_np, dtype=np.float32).reshape(B, C, N))
    prep = _host_prep_v2 if variant == "v2" else _host_prep
    W4, masks, foldrep = prep(np.asarray(covas_np, dtype=np.float32))
    in_maps = []
    for c in range(NCORES):
        in_maps.append({
            "q": np.ascontiguousarray(q[c * BPC:(c + 1) * BPC]),
            "w4": W4,
            "masks": masks,
            "foldrep": foldrep,
        })
    return in_maps


def assemble(results) -> np.ndarray:
    out = np.empty((B, K, N), np.float32)
    for c in range(NCORES):
        raw = results[c]["sim_raw"]                 # [BPC, 2, 128, 512]
        # raw[b, m, 32*s + k, f] -> sim[b, k, 1024*s + 512*m + f]
        r = raw.reshape(BPC, FPB // CHUNK, S, 32, CHUNK)[:, :, :, :K, :]
        out[c * BPC:(c + 1) * BPC] = (
            r.transpose(0, 3, 2, 1, 4).reshape(BPC, K, N))
    return np.ascontiguousarray(out.reshape(B, 1, K * N))


def _pick_variant(covas_np: np.ndarray) -> str:
    """v2 needs >=128 opposite-sign eigenvalue pairs across the K covas
    (always true for generic inputs); fall back to v1 otherwise."""
    total = 0
    for k in range(K):
        T = (covas_np[k].astype(np.float64) + covas_np[k].astype(np.float64).T) / 2
        lam = np.linalg.eigvalsh(T)
        total += min(int((lam > 0).sum()), int((lam <= 0).sum()))
    return "v2" if total >= 128 else "v1"


def kernel(input: np.ndarray, support_covas: np.ndarray) -> np.ndarray:
    covas = np.asarray(support_covas, dtype=np.float32)
    variant = _pick_variant(covas)
    nc = _get_nc(variant=variant)
    in_maps = make_in_maps(input, covas, variant=variant)
    res = bass_utils.run_bass_kernel_spmd(nc, in_maps, core_ids=list(range(NCORES)))
    return assemble(res.results)


if __name__ == "__main__":
    rng = np.random.default_rng(0)
    inp = rng.standard_normal((B, C, H, W)).astype(np.float32)
    cov = rng.standard_normal((K, C, C)).astype(np.float32)
    out = kernel(inp, cov)
    print("kernel output shape:", out.shape, out.dtype)
